# revision 15
# baseline (speedup 1.0000x reference)
"""DagEncoder (MLP + segment_sum) Trainium2 kernel, 8-core SPMD.

Contract: kernel(**inputs) takes the FULL unsharded inputs of
reference.setup_inputs() and returns the FULL [M, E] output.

Strategy (pure data parallelism over DAG segments):
  - 20000 segments split into 8 cores x 2500 segments; each core gets its
    node range. Within a core, segments are split into 2 "streams" so two
    nodes are processed per PE column (feature-major layout, 2x40 features
    stacked on partitions 0..79).
  - Host pads every segment to a multiple of 16 nodes (zero pad); node k
    of block b sits at column (b % 64) + k * 64 within chunk b // 64
    (chunks are 1024 columns).
  - Device per 1024-col chunk:
      mm1 (W1 blockdiag)  -> z1 psum [128, 1024]
      h1 = relu(z1 + b1)  -> split evac: ACT cols [0:A], DVE cols [A:1024]
      mm2 (W2 blockdiag)  -> z2 psum [128, 1024]
      h2lo = relu(z2[:, 0:512] + b2)          (ACT activation)
      g1 = max(z2[:, 512:1024], -b2) + h2lo   (DVE scalar_tensor_tensor)
      fold tree on GPSIMD: f_a (256), f_b (128), f_c -> parts [128, 64] f32
    parts accumulate per super-chunk and DMA to HBM.
  - Host: per-stream per-segment block sums via cumsum over the contiguous
    block range, exact pad/bias corrections, then @ W3 + b3.
"""

import sys
import types

sys.path.insert(0, "/opt/trn_rl_repo")

import numpy as np
import ml_dtypes

import concourse.bass as bass
import concourse.bacc as bacc
import concourse.mybir as mybir
import concourse.tile as tile
from concourse.bass_utils import run_bass_kernel_spmd

BF16 = ml_dtypes.bfloat16

NCORES = 8
B = 8            # nodes per block (segment padding unit)
FD = 1024        # psum chunk columns
KB = FD // B     # blocks per chunk (64)
SUPER = 4096     # DMA super-chunk columns
ACT_H1 = 608     # columns of each chunk's h1 evac done on ACT (rest on DVE)

# Stash of the last run's BassKernelResults for the dev harness.
LAST_RESULT = None


# ----------------------------------------------------------------------------
# Host-side layout
# ----------------------------------------------------------------------------

def _pack_stream(starts, cnts):
    """Sequential 16-node blocks for the segments of one stream.

    Returns (blk_src [nb] int64 node index of block start,
             blk_cnt [nb] int64 real nodes in block (1..16),
             seg_nb  [nsegs] int64 blocks per segment)."""
    nbs = -(-cnts // B)  # ceil, 0 for empty segments
    total = int(nbs.sum())
    blk_src = np.empty(total, np.int64)
    blk_cnt = np.empty(total, np.int64)
    # vectorized: for each segment, blocks j=0..nb-1 at starts + 16j
    seg_of_blk = np.repeat(np.arange(len(cnts)), nbs)
    j_in_seg = np.arange(total) - np.repeat(np.cumsum(nbs) - nbs, nbs)
    blk_src = starts[seg_of_blk] + j_in_seg * B
    blk_cnt = np.minimum(B, cnts[seg_of_blk] - j_in_seg * B)
    return blk_src, blk_cnt, nbs


def _node_src_for_cols(blk_src, blk_cnt, C):
    """node source index per column (-1 = pad): col j holds node
    k=(j%FD)//KB' ... node k of block b=(j//FD)*KB + (j%KB), k=(j%FD)//KB."""
    j = np.arange(C, dtype=np.int64)
    b = (j // FD) * KB + (j % KB)
    k = (j % FD) // KB
    src = blk_src[b] + k
    src = np.where((blk_src[b] >= 0) & (k < blk_cnt[b]), src, -1)
    return src


def _gather_T(a, src):
    """a[src].T with src == -1 rows zeroed; [a.shape[1], len(src)] bf16."""
    g = a[np.clip(src, 0, a.shape[0] - 1)]
    g[src < 0] = 0
    return np.ascontiguousarray(g.T)


def _split_streams(ptr, lo, hi):
    """Split segments [lo, hi) into two streams balancing padded blocks."""
    cnts = np.diff(ptr)[lo:hi]
    padded = -(-cnts // B)
    cum = np.concatenate([[0], np.cumsum(padded)])
    s = int(np.searchsorted(cum, cum[-1] // 2))
    s = min(max(s, 1), hi - lo - 1)
    return s


def _build_core_inputs(x, h_node, ptr, seg_lo, seg_hi, C):
    """xcat [80, C] bf16 and per-stream packing metadata."""
    xcat = np.zeros((80, C), BF16)
    meta = []
    s_split = _split_streams(ptr, seg_lo, seg_hi)
    for st in range(2):
        lo = seg_lo if st == 0 else seg_lo + s_split
        hi = seg_lo + s_split if st == 0 else seg_hi
        starts = ptr[lo:hi].astype(np.int64)
        cnts = np.diff(ptr)[lo:hi].astype(np.int64)
        blk_src, blk_cnt, seg_nb = _pack_stream(starts, cnts)
        nb = len(blk_src)
        assert nb * B <= C, (nb * B, C)
        bs = np.full(C // B, -1, np.int64)
        bc = np.zeros(C // B, np.int64)
        bs[:nb] = blk_src
        bc[:nb] = blk_cnt
        src = _node_src_for_cols(bs, bc, C)
        r0 = 40 * st
        xcat[r0:r0 + 8, :] = _gather_T(x, src)
        xcat[r0 + 8:r0 + 40, :] = _gather_T(h_node, src)
        meta.append(dict(lo=lo, hi=hi, seg_nb=seg_nb, blk_cnt=blk_cnt, nb=nb))
    return xcat, meta


# ----------------------------------------------------------------------------
# Device program
# ----------------------------------------------------------------------------

def _build_device_program(C):
    dt = mybir.dt
    AL = mybir.AluOpType
    ACTF = mybir.ActivationFunctionType
    NCH = C // FD           # chunks
    NSC = C // SUPER        # super-chunks
    CPS = SUPER // FD       # chunks per super-chunk
    NB = C // B             # block columns total

    LAG = 2                 # chunks mm2 trails mm1 by (h1 cold in SBUF)
    PC = 2 * KB             # parts columns per chunk (half-block sums)

    nc = bacc.Bacc(None, target_bir_lowering=False)

    xcat = nc.dram_tensor("xcat", [80, C], dt.bfloat16, kind="ExternalInput")
    w1 = nc.dram_tensor("w1blk", [80, 128], dt.bfloat16, kind="ExternalInput")
    w2 = nc.dram_tensor("w2blk", [128, 128], dt.bfloat16, kind="ExternalInput")
    b1s = nc.dram_tensor("b1s", [128, 1], dt.float32, kind="ExternalInput")
    b2s = nc.dram_tensor("b2s", [128, 1], dt.float32, kind="ExternalInput")
    nb2s = nc.dram_tensor("nb2s", [128, 1], dt.float32, kind="ExternalInput")
    outT = nc.dram_tensor("outT", [128, 2 * NB], dt.bfloat16,
                          kind="ExternalOutput")

    from contextlib import ExitStack

    with tile.TileContext(nc) as tc, ExitStack() as ctx:
        consts = ctx.enter_context(tc.tile_pool(name="consts", bufs=1))
        xin_pool = ctx.enter_context(tc.tile_pool(name="xin", bufs=3))
        h1_pool = ctx.enter_context(tc.tile_pool(name="h1", bufs=5))
        h2_pool = ctx.enter_context(tc.tile_pool(name="h2", bufs=3))
        out_pool = ctx.enter_context(tc.tile_pool(name="out", bufs=3))
        psum = ctx.enter_context(tc.tile_pool(name="psum", bufs=2, space="PSUM"))

        w1t = consts.tile([80, 128], dt.bfloat16)
        nc.sync.dma_start(w1t[:], w1[:])
        w2t = consts.tile([128, 128], dt.bfloat16)
        nc.sync.dma_start(w2t[:], w2[:])
        b1t = consts.tile([128, 1], dt.float32)
        nc.sync.dma_start(b1t[:], b1s[:])
        b2t = consts.tile([128, 1], dt.float32)
        nc.sync.dma_start(b2t[:], b2s[:])
        nb2t = consts.tile([128, 1], dt.float32)
        nc.sync.dma_start(nb2t[:], nb2s[:])

        xts = {}
        h1s = {}
        parts = {}

        def stage1(i):
            sc, cq = divmod(i, CPS)
            if cq == 0:
                xts[sc] = xin_pool.tile([80, SUPER], dt.bfloat16, tag="xt", name=f"xt_{sc}")
                nc.sync.dma_start(xts[sc][:],
                                  xcat[:, sc * SUPER:(sc + 1) * SUPER])
            xt = xts[sc]
            z1 = psum.tile([128, FD], dt.float32, tag="z1")
            nc.tensor.matmul(z1[:, 0:512], w1t[:],
                             xt[:, cq * FD:cq * FD + 512],
                             start=True, stop=True)
            nc.tensor.matmul(z1[:, 512:1024], w1t[:],
                             xt[:, cq * FD + 512:(cq + 1) * FD],
                             start=True, stop=True)
            h1 = h1_pool.tile([128, FD], dt.bfloat16, tag="h1")
            nc.scalar.activation(h1[:, 0:ACT_H1], z1[:, 0:ACT_H1],
                                 ACTF.Relu, bias=b1t[:], scale=1.0)
            nc.vector.tensor_scalar(h1[:, ACT_H1:FD], z1[:, ACT_H1:FD],
                                    b1t[:], 0.0, AL.add, AL.max)
            h1s[i] = h1

        def stage2(i):
            sc, cq = divmod(i, CPS)
            if cq == 0:
                parts[sc] = out_pool.tile([128, CPS * PC], dt.bfloat16,
                                          tag="parts", name=f"parts_{sc}")
            h1 = h1s.pop(i)
            z2 = psum.tile([128, FD], dt.float32, tag="z2")
            nc.tensor.matmul(z2[:, 0:512], w2t[:], h1[:, 0:512],
                             start=True, stop=True)
            nc.tensor.matmul(z2[:, 512:1024], w2t[:], h1[:, 512:1024],
                             start=True, stop=True)
            h2lo = h2_pool.tile([128, FD // 2], dt.bfloat16, tag="h2lo")
            nc.scalar.activation(h2lo[:], z2[:, 0:FD // 2],
                                 ACTF.Relu, bias=b2t[:], scale=1.0)
            g1 = h2_pool.tile([128, FD // 2], dt.bfloat16, tag="g1")
            nc.vector.scalar_tensor_tensor(g1[:], z2[:, FD // 2:FD],
                                           nb2t[:], h2lo[:],
                                           AL.max, AL.add)
            nc.gpsimd.tensor_tensor(parts[sc][:, cq * PC:(cq + 1) * PC],
                                    g1[:, 0:FD // 4],
                                    g1[:, FD // 4:FD // 2], op=AL.add)
            if cq == CPS - 1:
                nc.sync.dma_start(
                    outT[:, sc * CPS * PC:(sc + 1) * CPS * PC],
                    parts.pop(sc)[:])

        # pairs of chunks per weight set: mm1(i), mm1(i+1) share loaded w1;
        # mm2(i-2), mm2(i-1) share w2 — halves the weight-switch penalty.
        assert NCH % 2 == 0 and LAG == 2
        for i in range(0, NCH + LAG, 2):
            if i < NCH:
                stage1(i)
                stage1(i + 1)
            if i >= LAG:
                stage2(i - LAG)
                stage2(i - LAG + 1)

    nc.finalize()
    return nc


# ----------------------------------------------------------------------------
# Entry point
# ----------------------------------------------------------------------------

def _maybe_install_ntff_hook():
    try:
        import antenv.axon_hooks  # noqa: F401
        return
    except ImportError:
        pass
    try:
        from trn_agent_boot.trn_boot import _ntff_profile_via_ctypes
        hook = _ntff_profile_via_ctypes("/opt/axon/libaxon_pjrt.so")
        mod = types.ModuleType("antenv.axon_hooks")
        mod.get_axon_ntff_profile_hook = lambda: hook
        mod.set_axon_ntff_profile_hook = lambda h: None
        sys.modules["antenv.axon_hooks"] = mod
    except Exception:
        pass


def kernel(x, h_node, W1, b1, W2, b2, W3, b3, ptr):
    global LAST_RESULT
    x = np.asarray(x, np.float32)
    h_node = np.asarray(h_node, np.float32)
    W1 = np.asarray(W1, np.float32)
    W2 = np.asarray(W2, np.float32)
    W3 = np.asarray(W3, np.float32)
    b1 = np.asarray(b1, np.float32)
    b2 = np.asarray(b2, np.float32)
    b3 = np.asarray(b3, np.float32)
    ptr = np.asarray(ptr).astype(np.int64)
    N, F = x.shape
    E = h_node.shape[1]
    H = W1.shape[1]
    M = ptr.shape[0] - 1

    cnts = np.diff(ptr)

    # core boundaries: equalize padded-block counts (contiguous seg ranges)
    pb = -(-cnts // B)
    cum = np.concatenate([[0], np.cumsum(pb)])
    bounds = [0]
    for c in range(1, NCORES):
        t = int(np.searchsorted(cum, cum[-1] * c // NCORES))
        bounds.append(min(max(t, bounds[-1] + 1), M - (NCORES - c)))
    bounds.append(M)

    # common padded column count C: max padded blocks over core-streams
    nb_max = 0
    for c in range(NCORES):
        lo, hi = bounds[c], bounds[c + 1]
        s = _split_streams(ptr, lo, hi)
        for st in range(2):
            l2 = lo if st == 0 else lo + s
            h2_ = lo + s if st == 0 else hi
            nb = int(np.sum(-(-cnts[l2:h2_] // B)))
            nb_max = max(nb_max, nb)
    C = -(-nb_max * B // SUPER) * SUPER

    # device weights/constants
    w1blk = np.zeros((80, 128), np.float32)
    w1blk[0:40, 0:64] = W1
    w1blk[40:80, 64:128] = W1
    w2blk = np.zeros((128, 128), np.float32)
    w2blk[0:64, 0:64] = W2
    w2blk[64:128, 64:128] = W2
    b1st = np.concatenate([b1, b1]).reshape(128, 1).astype(np.float32)
    b2st = np.concatenate([b2, b2]).reshape(128, 1).astype(np.float32)

    in_maps = []
    metas = []
    for c in range(NCORES):
        xcat, meta = _build_core_inputs(x, h_node, ptr,
                                        bounds[c], bounds[c + 1], C)
        in_maps.append({
            "xcat": xcat,
            "w1blk": w1blk.astype(BF16),
            "w2blk": w2blk.astype(BF16),
            "b1s": b1st,
            "b2s": b2st,
            "nb2s": -b2st,
        })
        metas.append(meta)

    nc = _build_device_program(C)
    _maybe_install_ntff_hook()
    res = run_bass_kernel_spmd(nc, in_maps, core_ids=list(range(NCORES)))
    LAST_RESULT = res

    # ---- host assembly ----
    # Pad-column values: z1 = 0 -> h1_pad = relu(b1); v = W2 @ h1_pad.
    # lo half (k<8, ACT h2-form): pad contributes hpad = relu(v + b2)
    # hi half (k>=8, DVE g-form): pad contributes gpad = max(v, -b2)
    # real hi nodes contribute h2 - b2 (correct by +b2 per real hi node).
    h1p = np.maximum(b1, 0.0)
    v = h1p @ W2
    hpad = np.maximum(v + b2, 0.0)          # [H]
    gpad = np.maximum(v, -b2)               # [H]

    out = np.zeros((M, E), np.float32)
    for c in range(NCORES):
        raw = res.results[c]["outT"]        # [128, 2*NB] bf16 half-block sums
        # col j of chunk q: half j//64 of block q*64 + j%64
        r = raw.astype(np.float32).reshape(128, -1, 2, KB)
        parts = (r[:, :, 0, :] + r[:, :, 1, :]).reshape(128, -1)  # [128, NB]
        for st, m in enumerate(metas[c]):
            lo, hi = m["lo"], m["hi"]
            nsegs = hi - lo
            nb = m["nb"]
            seg_nb = m["seg_nb"]            # blocks per segment
            blk_cnt = m["blk_cnt"]          # real nodes per block
            p = parts[64 * st:64 * st + 64, :nb].T.astype(np.float32)  # [nb, H]
            # segment sums of block values via cumsum (safe for empty segs)
            csum = np.concatenate([np.zeros((1, H), np.float32),
                                   np.cumsum(p, axis=0)])
            ends = np.cumsum(seg_nb)
            begs = ends - seg_nb
            segdev = csum[ends] - csum[begs]            # [nsegs, H]
            # per-block pad counts -> per-segment totals (halves of B)
            HB = B // 2
            nreal_lo = np.minimum(blk_cnt, HB)
            nreal_hi = blk_cnt - nreal_lo
            npad_lo = HB - nreal_lo
            npad_hi = HB - nreal_hi
            z = np.zeros(1, np.int64)
            bc = np.concatenate([z, np.cumsum(npad_lo)])
            plo = (bc[ends] - bc[begs]).astype(np.float32)
            bc = np.concatenate([z, np.cumsum(npad_hi)])
            phi = (bc[ends] - bc[begs]).astype(np.float32)
            bc = np.concatenate([z, np.cumsum(nreal_hi)])
            rhi = (bc[ends] - bc[begs]).astype(np.float32)
            h2sum = (segdev
                     - plo[:, None] * hpad[None, :]
                     - phi[:, None] * gpad[None, :]
                     + rhi[:, None] * b2[None, :])
            out[lo:hi] = h2sum @ W3
    out += cnts[:, None].astype(np.float32) * b3[None, :]
    return out


# revision 17
# speedup vs baseline: 1.0043x; 1.0043x over previous
"""DagEncoder (MLP + segment_sum) Trainium2 kernel, 8-core SPMD.

Contract: kernel(**inputs) takes the FULL unsharded inputs of
reference.setup_inputs() and returns the FULL [M, E] output.

Strategy (pure data parallelism over DAG segments):
  - 20000 segments split into 8 cores x 2500 segments; each core gets its
    node range. Within a core, segments are split into 2 "streams" so two
    nodes are processed per PE column (feature-major layout, 2x40 features
    stacked on partitions 0..79).
  - Host pads every segment to a multiple of B=8 nodes (zero pad); node k
    of block b sits at column (b % 128) + k * 128 within chunk b // 128
    (chunks are 1024 columns). Core/stream boundaries are chosen to
    equalize padded-block counts.
  - Device per 1024-col chunk (mm2 lagged 2 chunks behind mm1):
      mm1 (W1 blockdiag)  -> z1 psum [128, 1024]
      h1 = relu(z1 + b1)  -> split evac: ACT cols [0:A], DVE cols [A:1024]
      mm2 (W2 blockdiag)  -> z2 psum [128, 1024]
      h2lo = relu(z2[:, 0:512] + b2)          (ACT activation, k<4)
      g1 = max(z2[:, 512:1024], -b2) + h2lo   (DVE scalar_tensor_tensor)
      parts = g1[:, 0:256] + g1[:, 256:512]   (GPSIMD; half-block sums)
    parts accumulate per super-chunk and DMA to HBM as bf16.
  - Host: fold half-block pairs, per-stream per-segment block sums via
    cumsum over the contiguous block range, exact pad/bias corrections
    (k<4 columns are h2-form, k>=4 are g-form), then @ W3 + b3.
"""

import sys
import types

sys.path.insert(0, "/opt/trn_rl_repo")

import numpy as np
import ml_dtypes

import concourse.bass as bass
import concourse.bacc as bacc
import concourse.mybir as mybir
import concourse.tile as tile
from concourse.bass_utils import run_bass_kernel_spmd

BF16 = ml_dtypes.bfloat16

NCORES = 8
B = 8            # nodes per block (segment padding unit)
FD = 1024        # psum chunk columns
KB = FD // B     # blocks per chunk (64)
SUPER = 4096     # DMA super-chunk columns
ACT_H1 = 608     # columns of each chunk's h1 evac done on ACT (rest on DVE)

# Stash of the last run's BassKernelResults for the dev harness.
LAST_RESULT = None


# ----------------------------------------------------------------------------
# Host-side layout
# ----------------------------------------------------------------------------

def _pack_stream(starts, cnts):
    """Sequential 16-node blocks for the segments of one stream.

    Returns (blk_src [nb] int64 node index of block start,
             blk_cnt [nb] int64 real nodes in block (1..16),
             seg_nb  [nsegs] int64 blocks per segment)."""
    nbs = -(-cnts // B)  # ceil, 0 for empty segments
    total = int(nbs.sum())
    blk_src = np.empty(total, np.int64)
    blk_cnt = np.empty(total, np.int64)
    # vectorized: for each segment, blocks j=0..nb-1 at starts + 16j
    seg_of_blk = np.repeat(np.arange(len(cnts)), nbs)
    j_in_seg = np.arange(total) - np.repeat(np.cumsum(nbs) - nbs, nbs)
    blk_src = starts[seg_of_blk] + j_in_seg * B
    blk_cnt = np.minimum(B, cnts[seg_of_blk] - j_in_seg * B)
    return blk_src, blk_cnt, nbs


def _node_src_for_cols(blk_src, blk_cnt, C):
    """node source index per column (-1 = pad): col j holds node
    k=(j%FD)//KB' ... node k of block b=(j//FD)*KB + (j%KB), k=(j%FD)//KB."""
    j = np.arange(C, dtype=np.int64)
    b = (j // FD) * KB + (j % KB)
    k = (j % FD) // KB
    src = blk_src[b] + k
    src = np.where((blk_src[b] >= 0) & (k < blk_cnt[b]), src, -1)
    return src


def _gather_T(a, src):
    """a[src].T with src == -1 rows zeroed; [a.shape[1], len(src)] bf16."""
    g = a[np.clip(src, 0, a.shape[0] - 1)]
    g[src < 0] = 0
    return np.ascontiguousarray(g.T)


def _split_streams(ptr, lo, hi):
    """Split segments [lo, hi) into two streams balancing padded blocks."""
    cnts = np.diff(ptr)[lo:hi]
    padded = -(-cnts // B)
    cum = np.concatenate([[0], np.cumsum(padded)])
    s = int(np.searchsorted(cum, cum[-1] // 2))
    s = min(max(s, 1), hi - lo - 1)
    return s


def _build_core_inputs(x, h_node, ptr, seg_lo, seg_hi, C):
    """xcat [80, C] bf16 and per-stream packing metadata."""
    xcat = np.zeros((80, C), BF16)
    meta = []
    s_split = _split_streams(ptr, seg_lo, seg_hi)
    for st in range(2):
        lo = seg_lo if st == 0 else seg_lo + s_split
        hi = seg_lo + s_split if st == 0 else seg_hi
        starts = ptr[lo:hi].astype(np.int64)
        cnts = np.diff(ptr)[lo:hi].astype(np.int64)
        blk_src, blk_cnt, seg_nb = _pack_stream(starts, cnts)
        nb = len(blk_src)
        assert nb * B <= C, (nb * B, C)
        bs = np.full(C // B, -1, np.int64)
        bc = np.zeros(C // B, np.int64)
        bs[:nb] = blk_src
        bc[:nb] = blk_cnt
        src = _node_src_for_cols(bs, bc, C)
        r0 = 40 * st
        xcat[r0:r0 + 8, :] = _gather_T(x, src)
        xcat[r0 + 8:r0 + 40, :] = _gather_T(h_node, src)
        meta.append(dict(lo=lo, hi=hi, seg_nb=seg_nb, blk_cnt=blk_cnt, nb=nb))
    return xcat, meta


# ----------------------------------------------------------------------------
# Device program
# ----------------------------------------------------------------------------

def _build_device_program(C):
    dt = mybir.dt
    AL = mybir.AluOpType
    ACTF = mybir.ActivationFunctionType
    NCH = C // FD           # chunks
    NSC = C // SUPER        # super-chunks
    CPS = SUPER // FD       # chunks per super-chunk
    NB = C // B             # block columns total

    LAG = 2                 # chunks mm2 trails mm1 by (h1 cold in SBUF)
    PC = 2 * KB             # parts columns per chunk (half-block sums)

    nc = bacc.Bacc(None, target_bir_lowering=False)

    xcat = nc.dram_tensor("xcat", [80, C], dt.bfloat16, kind="ExternalInput")
    w1 = nc.dram_tensor("w1blk", [80, 128], dt.bfloat16, kind="ExternalInput")
    w2 = nc.dram_tensor("w2blk", [128, 128], dt.bfloat16, kind="ExternalInput")
    b1s = nc.dram_tensor("b1s", [128, 1], dt.float32, kind="ExternalInput")
    b2s = nc.dram_tensor("b2s", [128, 1], dt.float32, kind="ExternalInput")
    nb2s = nc.dram_tensor("nb2s", [128, 1], dt.float32, kind="ExternalInput")
    outT = nc.dram_tensor("outT", [128, 2 * NB], dt.bfloat16,
                          kind="ExternalOutput")

    from contextlib import ExitStack

    with tile.TileContext(nc) as tc, ExitStack() as ctx:
        consts = ctx.enter_context(tc.tile_pool(name="consts", bufs=1))
        xin_pool = ctx.enter_context(tc.tile_pool(name="xin", bufs=3))
        h1_pool = ctx.enter_context(tc.tile_pool(name="h1", bufs=5))
        h2_pool = ctx.enter_context(tc.tile_pool(name="h2", bufs=3))
        out_pool = ctx.enter_context(tc.tile_pool(name="out", bufs=3))
        psum = ctx.enter_context(tc.tile_pool(name="psum", bufs=2, space="PSUM"))

        w1t = consts.tile([80, 128], dt.bfloat16)
        nc.sync.dma_start(w1t[:], w1[:])
        w2t = consts.tile([128, 128], dt.bfloat16)
        nc.sync.dma_start(w2t[:], w2[:])
        b1t = consts.tile([128, 1], dt.float32)
        nc.sync.dma_start(b1t[:], b1s[:])
        b2t = consts.tile([128, 1], dt.float32)
        nc.sync.dma_start(b2t[:], b2s[:])
        nb2t = consts.tile([128, 1], dt.float32)
        nc.sync.dma_start(nb2t[:], nb2s[:])

        xts = {}
        h1s = {}
        parts = {}

        def stage1(i):
            sc, cq = divmod(i, CPS)
            if cq == 0:
                xts[sc] = xin_pool.tile([80, SUPER], dt.bfloat16, tag="xt", name=f"xt_{sc}")
                nc.sync.dma_start(xts[sc][:],
                                  xcat[:, sc * SUPER:(sc + 1) * SUPER])
            xt = xts[sc]
            z1 = psum.tile([128, FD], dt.float32, tag="z1")
            nc.tensor.matmul(z1[:, 0:512], w1t[:],
                             xt[:, cq * FD:cq * FD + 512],
                             start=True, stop=True)
            nc.tensor.matmul(z1[:, 512:1024], w1t[:],
                             xt[:, cq * FD + 512:(cq + 1) * FD],
                             start=True, stop=True)
            h1 = h1_pool.tile([128, FD], dt.bfloat16, tag="h1")
            nc.scalar.activation(h1[:, 0:ACT_H1], z1[:, 0:ACT_H1],
                                 ACTF.Relu, bias=b1t[:], scale=1.0)
            nc.vector.tensor_scalar(h1[:, ACT_H1:FD], z1[:, ACT_H1:FD],
                                    b1t[:], 0.0, AL.add, AL.max)
            h1s[i] = h1

        def stage2(i):
            sc, cq = divmod(i, CPS)
            if cq == 0:
                parts[sc] = out_pool.tile([128, CPS * PC], dt.bfloat16,
                                          tag="parts", name=f"parts_{sc}")
            h1 = h1s.pop(i)
            z2 = psum.tile([128, FD], dt.float32, tag="z2")
            nc.tensor.matmul(z2[:, 0:512], w2t[:], h1[:, 0:512],
                             start=True, stop=True)
            nc.tensor.matmul(z2[:, 512:1024], w2t[:], h1[:, 512:1024],
                             start=True, stop=True)
            h2lo = h2_pool.tile([128, FD // 2], dt.bfloat16, tag="h2lo")
            nc.scalar.activation(h2lo[:], z2[:, 0:FD // 2],
                                 ACTF.Relu, bias=b2t[:], scale=1.0)
            g1 = h2_pool.tile([128, FD // 2], dt.bfloat16, tag="g1")
            nc.vector.scalar_tensor_tensor(g1[:], z2[:, FD // 2:FD],
                                           nb2t[:], h2lo[:],
                                           AL.max, AL.add)
            nc.gpsimd.tensor_tensor(parts[sc][:, cq * PC:(cq + 1) * PC],
                                    g1[:, 0:FD // 4],
                                    g1[:, FD // 4:FD // 2], op=AL.add)
            if cq == CPS - 1:
                nc.sync.dma_start(
                    outT[:, sc * CPS * PC:(sc + 1) * CPS * PC],
                    parts.pop(sc)[:])

        for i in range(NCH + LAG):
            if i < NCH:
                stage1(i)
            if i >= LAG:
                stage2(i - LAG)

    nc.finalize()
    return nc


# ----------------------------------------------------------------------------
# Entry point
# ----------------------------------------------------------------------------

def _maybe_install_ntff_hook():
    try:
        import antenv.axon_hooks  # noqa: F401
        return
    except ImportError:
        pass
    try:
        from trn_agent_boot.trn_boot import _ntff_profile_via_ctypes
        hook = _ntff_profile_via_ctypes("/opt/axon/libaxon_pjrt.so")
        mod = types.ModuleType("antenv.axon_hooks")
        mod.get_axon_ntff_profile_hook = lambda: hook
        mod.set_axon_ntff_profile_hook = lambda h: None
        sys.modules["antenv.axon_hooks"] = mod
    except Exception:
        pass


def kernel(x, h_node, W1, b1, W2, b2, W3, b3, ptr):
    global LAST_RESULT
    x = np.asarray(x, np.float32)
    h_node = np.asarray(h_node, np.float32)
    W1 = np.asarray(W1, np.float32)
    W2 = np.asarray(W2, np.float32)
    W3 = np.asarray(W3, np.float32)
    b1 = np.asarray(b1, np.float32)
    b2 = np.asarray(b2, np.float32)
    b3 = np.asarray(b3, np.float32)
    ptr = np.asarray(ptr).astype(np.int64)
    N, F = x.shape
    E = h_node.shape[1]
    H = W1.shape[1]
    M = ptr.shape[0] - 1

    cnts = np.diff(ptr)

    # core boundaries: equalize padded-block counts (contiguous seg ranges)
    pb = -(-cnts // B)
    cum = np.concatenate([[0], np.cumsum(pb)])
    bounds = [0]
    for c in range(1, NCORES):
        t = int(np.searchsorted(cum, cum[-1] * c // NCORES))
        bounds.append(min(max(t, bounds[-1] + 1), M - (NCORES - c)))
    bounds.append(M)

    # common padded column count C: max padded blocks over core-streams
    nb_max = 0
    for c in range(NCORES):
        lo, hi = bounds[c], bounds[c + 1]
        s = _split_streams(ptr, lo, hi)
        for st in range(2):
            l2 = lo if st == 0 else lo + s
            h2_ = lo + s if st == 0 else hi
            nb = int(np.sum(-(-cnts[l2:h2_] // B)))
            nb_max = max(nb_max, nb)
    C = -(-nb_max * B // SUPER) * SUPER

    # device weights/constants
    w1blk = np.zeros((80, 128), np.float32)
    w1blk[0:40, 0:64] = W1
    w1blk[40:80, 64:128] = W1
    w2blk = np.zeros((128, 128), np.float32)
    w2blk[0:64, 0:64] = W2
    w2blk[64:128, 64:128] = W2
    b1st = np.concatenate([b1, b1]).reshape(128, 1).astype(np.float32)
    b2st = np.concatenate([b2, b2]).reshape(128, 1).astype(np.float32)

    in_maps = []
    metas = []
    for c in range(NCORES):
        xcat, meta = _build_core_inputs(x, h_node, ptr,
                                        bounds[c], bounds[c + 1], C)
        in_maps.append({
            "xcat": xcat,
            "w1blk": w1blk.astype(BF16),
            "w2blk": w2blk.astype(BF16),
            "b1s": b1st,
            "b2s": b2st,
            "nb2s": -b2st,
        })
        metas.append(meta)

    nc = _build_device_program(C)
    _maybe_install_ntff_hook()
    res = run_bass_kernel_spmd(nc, in_maps, core_ids=list(range(NCORES)))
    LAST_RESULT = res

    # ---- host assembly ----
    # Pad-column values: z1 = 0 -> h1_pad = relu(b1); v = W2 @ h1_pad.
    # lo half (k<8, ACT h2-form): pad contributes hpad = relu(v + b2)
    # hi half (k>=8, DVE g-form): pad contributes gpad = max(v, -b2)
    # real hi nodes contribute h2 - b2 (correct by +b2 per real hi node).
    h1p = np.maximum(b1, 0.0)
    v = h1p @ W2
    hpad = np.maximum(v + b2, 0.0)          # [H]
    gpad = np.maximum(v, -b2)               # [H]

    out = np.zeros((M, E), np.float32)
    for c in range(NCORES):
        raw = res.results[c]["outT"]        # [128, 2*NB] bf16 half-block sums
        # col j of chunk q: half j//64 of block q*64 + j%64
        r = raw.astype(np.float32).reshape(128, -1, 2, KB)
        parts = (r[:, :, 0, :] + r[:, :, 1, :]).reshape(128, -1)  # [128, NB]
        for st, m in enumerate(metas[c]):
            lo, hi = m["lo"], m["hi"]
            nsegs = hi - lo
            nb = m["nb"]
            seg_nb = m["seg_nb"]            # blocks per segment
            blk_cnt = m["blk_cnt"]          # real nodes per block
            p = parts[64 * st:64 * st + 64, :nb].T.astype(np.float32)  # [nb, H]
            # segment sums of block values via cumsum (safe for empty segs)
            csum = np.concatenate([np.zeros((1, H), np.float32),
                                   np.cumsum(p, axis=0)])
            ends = np.cumsum(seg_nb)
            begs = ends - seg_nb
            segdev = csum[ends] - csum[begs]            # [nsegs, H]
            # per-block pad counts -> per-segment totals (halves of B)
            HB = B // 2
            nreal_lo = np.minimum(blk_cnt, HB)
            nreal_hi = blk_cnt - nreal_lo
            npad_lo = HB - nreal_lo
            npad_hi = HB - nreal_hi
            z = np.zeros(1, np.int64)
            bc = np.concatenate([z, np.cumsum(npad_lo)])
            plo = (bc[ends] - bc[begs]).astype(np.float32)
            bc = np.concatenate([z, np.cumsum(npad_hi)])
            phi = (bc[ends] - bc[begs]).astype(np.float32)
            bc = np.concatenate([z, np.cumsum(nreal_hi)])
            rhi = (bc[ends] - bc[begs]).astype(np.float32)
            h2sum = (segdev
                     - plo[:, None] * hpad[None, :]
                     - phi[:, None] * gpad[None, :]
                     + rhi[:, None] * b2[None, :])
            out[lo:hi] = h2sum @ W3
    out += cnts[:, None].astype(np.float32) * b3[None, :]
    return out
